# revision 26
# baseline (speedup 1.0000x reference)
"""DSS kernel on 8 trn2 cores — matmul-factorized.

out[l, h] = Re( sum_n Wk[h,n] * exp(dtL[h,n] * l) ),  (L=2048, H=1024)

Factor l = 64*l1 + l0 (l1 in [0,32), l0 in [0,64)):
  out[64*l1+l0, h] = sum_n Re(P[h,l1,n] * S0[h,n,l0])
                   = sum_n ReP*ReS0 - ImP*ImS0
with P = Wk * exp(dtL*64*l1), S0 = exp(dtL*l0).

Per channel this is one (32 x 128) @ (128 x 64) real matmul with the
128-row contraction = (n, re/im) stacked. The device does only PE
matmuls (stationary = S0 stack [128,64], moving = P stack [128,32],
psum out [64 l0, 32 l1]), a psum->sbuf f16 copy per 16-channel group,
and DMAs. Host does all transcendentals in f64 and ships K/M as fp16:
per core "km" [128, 8*1536] f16 (groups of 16 ch: 16*64 K cols then
16*32 M cols), out [64, 4096] f16 (col = i*32 + l1, row = l0).

Schedule (cost-model driven): 8 group-aligned input chunks on the SP
HWDGE queue saturate the DMA engines (~8.7us for 3MB); matmuls/copies
trail each chunk; out-DMAs also issue on SP AFTER all input DMAs in
program order so they never preempt the input stream on the FIFO DMA
device. Timeline ~= 2.0us startup + 8.7us input stream + ~5us tail
(chunk sem 0.9 + matmuls + copy + HWDGE 0.63 + DGE 0.65 + xfer + DMA
sem 0.9 + exit barrier).

Sharding: H split across 8 cores (128 channels each).
"""
import numpy as np

H, N, L_EXPECTED = 1024, 64, 2048
EPS = 1e-7
NCORES = 8
HC = H // NCORES          # 128 channels per core
P = 128                   # partitions (n, re/im stacked)
L1, L0 = 32, 64           # l = 64*l1 + l0
NG = 8                    # channel groups per core
GC = 16                   # channels per group
GCOLS = GC * L0 + GC * L1  # 1536 km cols per group
OC = GC * L1              # 512 out cols per group

_cache = {}


def _build_program():
    from contextlib import ExitStack
    from concourse import bacc, tile, mybir

    F32 = mybir.dt.float32
    F16 = mybir.dt.float16

    nc = bacc.Bacc("TRN2", target_bir_lowering=False, debug=False,
                   num_devices=NCORES)
    km_ap = nc.dram_tensor("km", [P, NG * GCOLS], F16, kind="ExternalInput").ap()
    out_ap = nc.dram_tensor("out", [L0, NG * OC], F16, kind="ExternalOutput").ap()

    with tile.TileContext(nc) as tc, ExitStack() as ctx:
        km_pool = ctx.enter_context(tc.tile_pool(name="km", bufs=1))
        o_pool = ctx.enter_context(tc.tile_pool(name="o", bufs=1))
        ps_pool = ctx.enter_context(tc.tile_pool(name="ps", bufs=1, space="PSUM"))

        km_t = km_pool.tile([P, NG * GCOLS], F16, tag="km")
        # Channel-interleaved layout (each channel's K then M contiguous)
        # lets the last group arrive as two half-chunks: its first 8
        # channels' matmuls and half-copy complete before the final chunk's
        # DMA semaphore even fires.
        CW = L0 + L1  # 96 cols per channel
        bounds = [g * GCOLS for g in range(NG)] + [NG * GCOLS - GCOLS // 2,
                                                   NG * GCOLS]
        for ci in range(len(bounds) - 1):
            nc.sync.dma_start(km_t[:, bounds[ci]:bounds[ci + 1]],
                              km_ap[:, bounds[ci]:bounds[ci + 1]])

        for g in range(NG):
            base = g * GCOLS
            ps = ps_pool.tile([L0, OC], F32, tag=f"ps{g}", name=f"ps{g}")
            for i2 in range(GC):
                lhsT = km_t[:, base + i2 * CW: base + i2 * CW + L0]
                rhs = km_t[:, base + i2 * CW + L0: base + (i2 + 1) * CW]
                nc.tensor.matmul(ps[:, i2 * L1:(i2 + 1) * L1], lhsT, rhs,
                                 start=True, stop=True)
            ot = o_pool.tile([L0, OC], F16, tag=f"o{g}", name=f"o{g}")
            if g < NG - 1:
                nc.vector.tensor_copy(ot[:], ps[:])
            else:
                nc.vector.tensor_copy(ot[:, :OC // 2], ps[:, :OC // 2])
                nc.vector.tensor_copy(ot[:, OC // 2:], ps[:, OC // 2:])
            nc.sync.dma_start(out_ap[:, g * OC:(g + 1) * OC], ot[:])
    nc.compile()
    return nc


def _prep_inputs(log_dt, llnr, lim, W):
    """All f64 host prep. Returns per-core input dicts."""
    LamRe = -np.exp(llnr.astype(np.float64))          # (N,)
    LamIm = lim.astype(np.float64)                    # (N,)
    Lam = LamRe + 1j * LamIm
    dt = np.exp(log_dt.astype(np.float64))            # (H,2)
    a = dt[:, 0:1] * LamRe[None, :]                   # (H,N)
    b = dt[:, 1:2] * LamIm[None, :]                   # (H,N)
    dtL = a + 1j * b
    Wc = W[..., 0].astype(np.float64) + 1j * W[..., 1].astype(np.float64)
    norm_sq = np.maximum((Lam * np.conj(Lam)).real, EPS * EPS)
    recip = np.conj(Lam) / norm_sq
    Wk = Wc * (np.exp(dtL) - 1.0) * recip[None, :]    # (H,N) complex

    l0 = np.arange(L0, dtype=np.float64)
    l1 = np.arange(L1, dtype=np.float64)
    # S0[h, n, l0] = exp(dtL*l0); K stacks (Re; Im) on partitions.
    s0 = np.exp(dtL[:, :, None] * l0[None, None, :])        # (H,N,L0) complex
    # P[h, l1, n] = Wk * exp(dtL*64*l1); M stacks (Re; -Im).
    pmat = Wk[:, None, :] * np.exp(dtL[:, None, :] * (64.0 * l1)[None, :, None])

    in_maps = []
    for core in range(NCORES):
        h0 = core * HC
        kre = s0.real[h0:h0 + HC].transpose(1, 0, 2)        # (N, HC, L0)
        kim = s0.imag[h0:h0 + HC].transpose(1, 0, 2)
        mre = pmat.real[h0:h0 + HC].transpose(2, 0, 1)      # (N, HC, L1)
        mim = -pmat.imag[h0:h0 + HC].transpose(2, 0, 1)
        km = np.empty((P, NG, GC, L0 + L1), np.float16)
        kmK = km[:, :, :, :L0]
        kmM = km[:, :, :, L0:]
        kmK[:N] = kre.reshape(N, NG, GC, L0)
        kmK[N:] = kim.reshape(N, NG, GC, L0)
        kmM[:N] = mre.reshape(N, NG, GC, L1)
        kmM[N:] = mim.reshape(N, NG, GC, L1)
        in_maps.append(dict(km=km.reshape(P, NG * GCOLS)))
    return in_maps


def _reference_numpy(log_dt, llnr, lim, W, L):
    """f32 fallback for unexpected L (matches reference.py semantics)."""
    Lam = -np.exp(llnr.astype(np.float32)) + 1j * lim.astype(np.float32)
    Wc = W[..., 0] + 1j * W[..., 1]
    dt = np.exp(log_dt.astype(np.float32))
    dtL = dt[:, 0:1] * Lam.real + 1j * (dt[:, 1:2] * Lam.imag)
    pos = np.arange(L, dtype=np.float32)
    S = np.exp(dtL[None, :, :] * pos[:, None, None])
    norm_sq = np.maximum((Lam * np.conj(Lam)).real, np.float32(EPS * EPS))
    Wk = Wc * (np.exp(dtL) - 1.0) * (np.conj(Lam) / norm_sq)
    return np.einsum('hn,lhn->lh', Wk, S).real.astype(np.float32)


def kernel(**inputs):
    log_dt = np.asarray(inputs["log_dt"], np.float32)
    llnr = np.asarray(inputs["Lambda_log_neg_re"], np.float32)
    lim = np.asarray(inputs["Lambda_im"], np.float32)
    W = np.asarray(inputs["W"], np.float32)
    L = int(inputs["L"])

    if L != L_EXPECTED or log_dt.shape != (H, 2) or W.shape != (H, N, 2):
        return _reference_numpy(log_dt, llnr, lim, W, L)

    from concourse.bass_utils import run_bass_kernel_spmd

    if "nc" not in _cache:
        _cache["nc"] = _build_program()
    nc = _cache["nc"]

    in_maps = _prep_inputs(log_dt, llnr, lim, W)
    res = run_bass_kernel_spmd(nc, in_maps, core_ids=list(range(NCORES)))
    out = np.empty((L_EXPECTED, H), np.float32)
    for core in range(NCORES):
        arr = res.results[core]["out"]                       # (L0, HC*L1)
        arr = np.asarray(arr, np.float32).reshape(L0, HC, L1)
        out[:, core * HC:(core + 1) * HC] = (
            arr.transpose(2, 0, 1).reshape(L_EXPECTED, HC))
    return out
